# revision 1
# baseline (speedup 1.0000x reference)
"""Multi-head attention with relative-position-bias MLP on 8 TRN2 NeuronCores.

Strategy: pure data-parallel over batch (B=8 -> 1 batch element per core, no
collectives). Host-side prep is layout only: per-core transposed x (plus a
token-reversed copy feeding k/v), transposed weights, replicated proj bias,
and exp() of the 63x63 relative-position bias table (the bias is a
2D-Toeplitz expansion of a tiny MLP on 63*63 distinct (rel_x, rel_y) points;
~7 MFLOP of a 66 GFLOP problem).

Device algorithm per core (N=1024 tokens, C=768, H=12 heads, D=64):
  qT[o,n] = qkv_wT[:, o].T @ xT
  kT[o,n] = qkv_wT[:, o].T @ xRT      (token-reversed k)
  v[n,o]  = xRT.T @ qkv_wT[:, v-sec]  (token-reversed v, + ones column)
  per head pair (2j, 2j+1), k-tile t (128 reversed tokens):
     sT_h = kT_h(t).T @ qT_h          [nk=128, nq=1024] (pair interleaved so
                                       the two K=64 matmuls overlap in PE)
     E = exp(sT/8)                    (ACT, scale folded into exp)
     P = E * expB_tile                (DVE/GPSIMD alternating, all-SBUF;
                                       exp(s+b) = exp(s)*exp(b))
     av[h,c] += [v_h(t) | 1].T @ P    (PE accumulate; row 64 = colsum)
  outT_h(c) = av[0:64] * recip(av[64])  (recip replicated via K=1 f32r MM)
  final = outT.T @ proj_wT (+ proj_b via K=1 MM accumulate)

Token reversal trick: bias[h,n,m] depends on grid coords of (n,m) only via
(cy_n - cy_m, cx_n - cx_m). Reversing key/value token order makes the
Toeplitz expansion all-positive-stride: TBLREP_h[p, J] = expG_h[63*(p//32)
+ p%32 + J] (4 plain DMAs per head), and each [128,1024] bias tile is a
strided view of it. The AV reduction over k-tiles is order-invariant.
"""
import sys

import numpy as np

sys.path.insert(0, "/opt/trn_rl_repo")

import concourse.bass as bass  # noqa: E402
import concourse.mybir as mybir  # noqa: E402
import concourse.tile as tile  # noqa: E402
from concourse import bacc  # noqa: E402
from concourse.bass_utils import run_bass_kernel_spmd  # noqa: E402

F32 = mybir.dt.float32
F32R = mybir.dt.float32r
BF16 = mybir.dt.bfloat16
EXP = mybir.ActivationFunctionType.Exp
COPY = mybir.ActivationFunctionType.Copy

B, N, C, H, D = 8, 1024, 768, 12, 64
SCALE = float(D) ** -0.5
NT = N // 128   # 8 token tiles
CT = C // 128   # 6 channel tiles
TBLW = 3781     # TBLREP width (padded so 2016-wide views stay in range)
TW = 4001       # DRAM table width per head (>= 220 + TBLW, zero-padded)


def _build_graph():
    nc = bacc.Bacc("TRN2", target_bir_lowering=False, debug=False,
                   enable_asserts=False, num_devices=B)
    xT_d = nc.dram_tensor("xT", [C, N], F32, kind="ExternalInput")
    xRT_d = nc.dram_tensor("xRT", [C, N], F32, kind="ExternalInput")
    wqkv_d = nc.dram_tensor("qkv_wT", [C, 3 * C], F32, kind="ExternalInput")
    wproj_d = nc.dram_tensor("proj_wT", [C, C], F32, kind="ExternalInput")
    pbrep_d = nc.dram_tensor("proj_b_rep", [128, C], F32, kind="ExternalInput")
    tbl_d = nc.dram_tensor("rpb_tbl", [H, TW], BF16, kind="ExternalInput")
    out_d = nc.dram_tensor("out", [N, C], F32, kind="ExternalOutput")

    with tile.TileContext(nc) as tc:
        _kern(tc, nc, xT_d, xRT_d, wqkv_d, wproj_d, pbrep_d, tbl_d, out_d)
    nc.compile()
    return nc


def _kern(tc, nc, xT_d, xRT_d, wqkv_d, wproj_d, pbrep_d, tbl_d, out_d):
    from contextlib import ExitStack

    with ExitStack() as es:
        persist = es.enter_context(tc.tile_pool(name="persist", bufs=1))
        # qT tiles 0..5, kT tiles 6..11; [o-part, n-free]
        qk_sb = [persist.tile([128, N], F32R, tag=f"qk{i}", name=f"qk{i}")
                 for i in range(12)]
        # v (token-reversed) head-strided with ones column at h*65+64
        vaug = [persist.tile([128, H * 65], BF16, tag=f"va{i}", name=f"va{i}")
                for i in range(NT)]
        # attention output transposed [c, n], c = h*64+d
        outT = [persist.tile([128, N], F32R, tag=f"ot{i}", name=f"ot{i}")
                for i in range(CT)]
        ones_f = persist.tile([128, 64], F32, tag="onesf")
        nc.vector.memset(ones_f[:], 1.0)
        ones_r = persist.tile([128, 128], F32R, tag="onesr")
        nc.vector.tensor_copy(ones_r[:, 0:64], ones_f[:])
        nc.vector.tensor_copy(ones_r[:, 64:128], ones_f[:])
        onescol = persist.tile([128, H], F32, tag="onescol")
        nc.vector.memset(onescol[:], 1.0)
        for t in range(NT):
            va_v = vaug[t][:].rearrange("p (h e) -> p h e", e=65)
            nc.vector.tensor_copy(va_v[:, :, 64:65], onescol[:].unsqueeze(-1))

        # ---------------- QKV ----------------
        with ExitStack() as esq:
            ld = esq.enter_context(tc.tile_pool(name="ld", bufs=1))
            xT = [ld.tile([128, N], F32R, tag=f"x{i}", name=f"x{i}")
                  for i in range(CT)]
            xRT = [ld.tile([128, N], F32R, tag=f"xr{i}", name=f"xr{i}")
                   for i in range(CT)]
            wq = [ld.tile([128, C], F32R, tag=f"wq{i}", name=f"wq{i}")
                  for i in range(CT)]
            wk = [ld.tile([128, C], F32R, tag=f"wk{i}", name=f"wk{i}")
                  for i in range(CT)]
            for i in range(CT):
                nc.gpsimd.dma_start(xT[i][:], xT_d.ap()[i * 128:(i + 1) * 128, :])
                nc.gpsimd.dma_start(xRT[i][:],
                                    xRT_d.ap()[i * 128:(i + 1) * 128, :])
                nc.gpsimd.dma_start(wq[i][:],
                                    wqkv_d.ap()[i * 128:(i + 1) * 128, 0:C])
                nc.gpsimd.dma_start(wk[i][:],
                                    wqkv_d.ap()[i * 128:(i + 1) * 128, C:2 * C])
            qps = esq.enter_context(tc.tile_pool(name="qps", bufs=4, space="PSUM"))
            for ot in range(12):
                rhs_src = xT if ot < 6 else xRT
                for c in range(2):
                    ps = qps.tile([128, 512], F32, tag="ps", name="qkps", bufs=6)
                    wsrc = wq if ot < 6 else wk
                    oo = (ot % 6) * 128
                    for kt in range(CT):
                        nc.tensor.matmul(
                            ps[:], wsrc[kt][:, oo:oo + 128],
                            rhs_src[kt][:, c * 512:(c + 1) * 512],
                            start=(kt == 0), stop=(kt == CT - 1))
                    nc.vector.tensor_copy(qk_sb[ot][:, c * 512:(c + 1) * 512],
                                          ps[:])
            # v from xRT in natural layout [n, o], head-strided into vaug
            wv = [ld.tile([128, C], F32R, tag=f"w{i}", name=f"wv{i}")
                  for i in range(CT)]
            for i in range(CT):
                nc.gpsimd.dma_start(
                    wv[i][:], wqkv_d.ap()[i * 128:(i + 1) * 128, 2 * C:3 * C])
            for t in range(NT):
                for vc in range(2):
                    ps = qps.tile([128, 384], F32, tag="psv", name="vps", bufs=2)
                    for kt in range(CT):
                        nc.tensor.matmul(
                            ps[:], xRT[kt][:, t * 128:(t + 1) * 128],
                            wv[kt][:, vc * 384:(vc + 1) * 384],
                            start=(kt == 0), stop=(kt == CT - 1))
                    va_v = vaug[t][:].rearrange("p (h e) -> p h e", e=65)
                    ps_v = ps[:].rearrange("p (h d) -> p h d", d=64)
                    nc.vector.tensor_copy(va_v[:, vc * 6:(vc + 1) * 6, 0:64], ps_v)

        # ---------------- attention + proj ----------------
        with ExitStack() as esr:
            ldp = esr.enter_context(tc.tile_pool(name="ldp", bufs=1))
            pwT = [ldp.tile([128, C], F32R, tag=f"pw{i}", name=f"pw{i}")
                   for i in range(CT)]
            pbrow = ldp.tile([128, C], F32R, tag="pbrow")
            for i in range(CT):
                nc.gpsimd.dma_start(pwT[i][:],
                                    wproj_d.ap()[i * 128:(i + 1) * 128, :])
            nc.gpsimd.dma_start(pbrow[:], pbrep_d.ap()[:, :])

            with ExitStack() as esa:
                tblp = esa.enter_context(tc.tile_pool(name="tblp", bufs=3))
                ep = esa.enter_context(tc.tile_pool(name="expp", bufs=10))
                pp = esa.enter_context(tc.tile_pool(name="phat", bufs=10))
                tmpp = esa.enter_context(tc.tile_pool(name="tmp", bufs=4))
                sps = esa.enter_context(
                    tc.tile_pool(name="sps", bufs=2, space="PSUM"))
                avps = esa.enter_context(
                    tc.tile_pool(name="avps", bufs=4, space="PSUM"))

                for j in range(H // 2):
                    hpair = (2 * j, 2 * j + 1)
                    # TBLREP per head: TBL[p, J] = expG_h[63*(p//32)+p%32+J]
                    tbls = []
                    for h in hpair:
                        tblt = tblp.tile([128, TBLW], BF16, tag="tbl",
                                         name=f"tbl{h}")
                        for blk in range(4):
                            eng = nc.gpsimd if blk % 2 == 0 else nc.sync
                            eng.dma_start(
                                tblt[blk * 32:(blk + 1) * 32, :],
                                bass.AP(tbl_d, h * TW + 63 * blk,
                                        [[1, 32], [1, TBLW]]))
                        tbls.append(tblt)
                    avs = {h: [avps.tile([65, 512], F32, tag="av",
                                         name=f"av{h}_{c}") for c in range(2)]
                           for h in hpair}
                    for t in range(NT):
                        pss = [sps.tile([128, 1024], F32, tag="sc",
                                        name=f"sc{h}_{t}") for h in hpair]
                        # interleave the two heads' K=64 matmuls so they
                        # overlap in the PE array (row groups 0-1 vs 2-3)
                        for c in range(2):
                            for hi, h in enumerate(hpair):
                                qh = qk_sb[j][(h % 2) * 64:(h % 2) * 64 + 64, :]
                                kh = qk_sb[6 + j][(h % 2) * 64:(h % 2) * 64 + 64, :]
                                nc.tensor.matmul(
                                    pss[hi][:, c * 512:(c + 1) * 512],
                                    kh[:, t * 128:(t + 1) * 128],
                                    qh[:, c * 512:(c + 1) * 512],
                                    start=True, stop=True)
                        for hi, h in enumerate(hpair):
                            ee = ep.tile([128, 1024], BF16, tag="ee",
                                         name=f"ee{h}_{t}")
                            nc.scalar.activation(ee[:], pss[hi][:], EXP,
                                                 scale=SCALE)
                            tv = tbls[hi][:, 252 * t:252 * t + 2016].rearrange(
                                "p (c a b) -> p c a b", c=2, b=63)[:, :, :, :32]
                            ph = pp.tile([128, 1024], BF16, tag="ph",
                                         name=f"ph{h}_{t}")
                            pv = ph[:].rearrange("p (c a b) -> p c a b",
                                                 c=2, b=32)
                            ev = ee[:].rearrange("p (c a b) -> p c a b",
                                                 c=2, b=32)
                            nc.vector.tensor_mul(pv, ev, tv)
                            for c in range(2):
                                nc.tensor.matmul(
                                    avs[h][c][:],
                                    vaug[t][:, h * 65:(h + 1) * 65],
                                    ph[:, c * 512:(c + 1) * 512],
                                    start=(t == 0), stop=(t == NT - 1))
                    for h in hpair:
                        for c in range(2):
                            avsb = tmpp.tile([65, 512], F32, tag="avsb",
                                             name=f"avsb{h}{c}")
                            nc.vector.tensor_copy(avsb[:], avs[h][c][:])
                            rsb = tmpp.tile([128, 512], F32R, tag="rsb",
                                            name=f"rsb{h}{c}")
                            with nc.allow_low_precision(
                                    reason="softmax recip rounded to f32r"):
                                nc.vector.reciprocal(rsb[64:65, :],
                                                     avsb[64:65, :])
                            rep = avps.tile([64, 512], F32, tag="av",
                                            name=f"rep{h}{c}")
                            nc.tensor.matmul(rep[:], ones_r[64:65, 0:64],
                                             rsb[64:65, :],
                                             start=True, stop=True)
                            dst = outT[h // 2][(h % 2) * 64:(h % 2) * 64 + 64,
                                               c * 512:(c + 1) * 512]
                            if h % 2 == 0:
                                nc.vector.tensor_mul(dst, avsb[0:64, :],
                                                     rep[:])
                            else:
                                tmp = tmpp.tile([64, 512], F32R, tag="tmo",
                                                name=f"tmo{h}{c}")
                                nc.vector.tensor_mul(tmp[:], avsb[0:64, :],
                                                     rep[:])
                                nc.sync.dma_start(dst, tmp[:])

            # ---------------- proj ----------------
            with ExitStack() as esp:
                pjps = esp.enter_context(
                    tc.tile_pool(name="pjps", bufs=4, space="PSUM"))
                fsb = esp.enter_context(tc.tile_pool(name="fsb", bufs=4))
                for t in range(NT):
                    f = fsb.tile([128, C], F32, tag="f", name=f"f{t}")
                    for pc in range(2):
                        ps = pjps.tile([128, 384], F32, tag="ps", name="pjps")
                        for kt in range(CT):
                            nc.tensor.matmul(
                                ps[:], outT[kt][:, t * 128:(t + 1) * 128],
                                pwT[kt][:, pc * 384:(pc + 1) * 384],
                                start=(kt == 0), stop=False)
                        # + proj bias via K=1 matmul accumulate
                        nc.tensor.matmul(
                            ps[:], ones_r[0:1, 0:128],
                            pbrow[0:1, pc * 384:(pc + 1) * 384],
                            start=False, stop=True)
                        nc.vector.tensor_copy(f[:, pc * 384:(pc + 1) * 384],
                                              ps[:])
                    nc.sync.dma_start(out_d.ap()[t * 128:(t + 1) * 128, :], f[:])


_GRAPH = None


def _graph():
    global _GRAPH
    if _GRAPH is None:
        _GRAPH = _build_graph()
    return _GRAPH


def _host_prep(x, qkv_w, proj_w, proj_b, rpb_w1, rpb_b1, rpb_w2, rpb_b2):
    """Numpy layout prep + exp of the 63x63 bias table (7 MFLOP)."""
    a = np.arange(63, dtype=np.float32) - 31.0
    rel_y = np.broadcast_to(a[:, None], (63, 63))
    rel_x = np.broadcast_to(a[None, :], (63, 63))
    rel = np.stack([rel_x, rel_y], -1).reshape(-1, 2)           # [3969, 2]
    hdn = np.maximum(rel @ rpb_w1.T + rpb_b1, 0.0)
    gtbl = (hdn @ rpb_w2.T + rpb_b2).T.astype(np.float32)       # [12, 3969]
    gtbl = np.exp(gtbl, dtype=np.float32)                       # exp(bias)
    import ml_dtypes
    gpad = np.zeros((H, TW), np.float32)
    gpad[:, :3969] = gtbl
    gpad = gpad.astype(ml_dtypes.bfloat16)

    wqkvT = np.ascontiguousarray(qkv_w.T.astype(np.float32))    # [768, 2304]
    wprojT = np.ascontiguousarray(proj_w.T.astype(np.float32))  # [768, 768]
    pbrep = np.ascontiguousarray(
        np.broadcast_to(proj_b.astype(np.float32), (128, C)))
    shared = {"qkv_wT": wqkvT, "proj_wT": wprojT, "proj_b_rep": pbrep,
              "rpb_tbl": gpad}
    in_maps = []
    for i in range(B):
        m = dict(shared)
        m["xT"] = np.ascontiguousarray(x[i].T.astype(np.float32))
        m["xRT"] = np.ascontiguousarray(x[i][::-1].T.astype(np.float32))
        in_maps.append(m)
    return in_maps


def kernel(x, qkv_w, proj_w, proj_b, rpb_w1, rpb_b1, rpb_w2, rpb_b2,
           _trace=False, _tmpdir=None):
    in_maps = _host_prep(np.asarray(x), np.asarray(qkv_w), np.asarray(proj_w),
                         np.asarray(proj_b), np.asarray(rpb_w1),
                         np.asarray(rpb_b1), np.asarray(rpb_w2),
                         np.asarray(rpb_b2))
    nc = _graph()
    res = run_bass_kernel_spmd(nc, in_maps, core_ids=list(range(B)),
                               trace=_trace, tmpdir=_tmpdir)
    out = np.stack([res.results[i]["out"] for i in range(B)])
    if _trace:
        kernel._last_results = res
    return out

